# revision 2
# baseline (speedup 1.0000x reference)
"""Trainium2 Bass kernel for nn_CNN2DImplemented_51994874085714.

conv2d: x (16, 64, 112, 112) f32 * weight (64, 3, 3, 128) -> (16, 128, 112, 112),
3x3, pad=1, stride=1 (weight layout (C_in, kh, kw, C_out), no bias).

Sharding: data-parallel over batch - 2 images per NeuronCore on 8 cores,
weight replicated; each core computes its shard independently (no
collectives) and the host concatenates the per-core outputs.

x and weight are converted to bf16 on the host before upload: bf16 matmuls
run at the same 1 column/cycle PE rate as fp32r but halve the input DMA
traffic and SBUF footprint (measured ~3% faster end-to-end on HW, rel err
~2.5e-3, well inside the 2e-2 gate). Output stays f32: an f32->bf16 drain
on the DVE measured pathologically slow on HW, and bf16 stores bought
nothing once drains moved engines (the kernel is PE-stream bound).

Per-core kernel (implicit GEMM), per 28-row strip task:
  one strided DMA into a padded [128, 31, 114] bf16 tile (p0:64 = x_pad
  rows, zero pad columns), one GPSIMD row-shift copy building the dh=1
  frame on p64:128, then per 4-row block six K=128 matmuls of N=448
  accumulating one PSUM bank:
    pair MM  (dh=0,1): lhsT = [W[:,0,dw,:]; W[:,1,dw,:]]   dw = 0,1,2
    single MM (dh=2):  lhsT = [W[:,2,dw,:]; 0]             dw = 0,1,2
  DVE copies banks to an f32 SBUF staging strip; one packed DMA stores it.
"""

from contextlib import ExitStack

import numpy as np

N_CORES = 8
B, C, H, W, O = 16, 64, 112, 112, 128
B_LOC = B // N_CORES
S = 28  # output rows per strip

_cache = {}


def _build_nc():
    import concourse.mybir as mybir
    import concourse.tile as tile
    from concourse import bacc

    F32 = mybir.dt.float32
    BF16 = mybir.dt.bfloat16

    nc = bacc.Bacc("TRN2", target_bir_lowering=False, debug=False,
                   num_devices=N_CORES)
    x_d = nc.declare_dram_parameter("x", [B_LOC, C, H, W], BF16, isOutput=False)
    w_d = nc.declare_dram_parameter("weight", [C, 3, 3, O], BF16, isOutput=False)
    o_d = nc.declare_dram_parameter("out", [B_LOC, O, H, W], F32, isOutput=True)

    R = S + 3
    Wp = W + 2
    NS = H // S

    with tile.TileContext(nc) as tc, ExitStack() as ctx:
        wpool = ctx.enter_context(tc.tile_pool(name="weights", bufs=1))
        xpool = ctx.enter_context(tc.tile_pool(name="xstrips", bufs=4))
        spool = ctx.enter_context(tc.tile_pool(name="staging", bufs=3))
        ppool = ctx.enter_context(tc.tile_pool(name="psum", bufs=6, space="PSUM"))

        zrow = wpool.tile([64, O], F32, tag="zrow")
        nc.vector.memset(zrow[:, :], 0.0)
        wpair = []
        wsing = []
        for dw in range(3):
            wp = wpool.tile([128, O], BF16, tag=f"wpair{dw}")
            ws = wpool.tile([128, O], BF16, tag=f"wsing{dw}")
            nc.sync.dma_start(wp[0:64, :], w_d[:, 0, dw, :])
            nc.sync.dma_start(wp[64:128, :], w_d[:, 1, dw, :])
            nc.sync.dma_start(ws[0:64, :], w_d[:, 2, dw, :])
            nc.vector.tensor_copy(ws[64:128, :], zrow[:, :])
            wpair.append(wp)
            wsing.append(ws)

        def load_task(t):
            s, img = t
            h0 = s * S
            xb = xpool.tile([128, R, Wp], BF16, tag="xs")
            nc.vector.memset(xb[0:64, :, 0], 0.0)
            nc.vector.memset(xb[0:64, :, Wp - 1], 0.0)
            r_lo = max(0, 1 - h0)
            r_hi = min(S + 2, H - h0)
            if r_lo > 0:
                nc.vector.memset(xb[0:64, 0:r_lo, :], 0.0)
            if r_hi < S + 2:
                nc.vector.memset(xb[0:64, r_hi + 1:S + 3, :], 0.0)
            nc.sync.dma_start(
                xb[0:64, r_lo:r_hi + 1, 1:W + 1],
                x_d[img, :, h0 + r_lo - 1:h0 + r_hi, :],
            )
            nc.gpsimd.tensor_copy(xb[64:128, 0:S + 2, :], xb[0:64, 1:S + 3, :])
            return xb

        def compute(s, img, xb):
            h0 = s * S
            stg = spool.tile([O, S, W], F32, tag="stg")
            for j in range(S // 4):
                l0 = 4 * j
                ps = ppool.tile([O, 4, W], F32, tag="ps")
                for dw in range(3):
                    nc.tensor.matmul(
                        ps[:, :, :],
                        wpair[dw][:, :],
                        xb[:, l0:l0 + 4, dw:dw + W],
                        start=(dw == 0), stop=False,
                    )
                for dw in range(3):
                    nc.tensor.matmul(
                        ps[:, :, :],
                        wsing[dw][:, :],
                        xb[:, l0 + 2:l0 + 6, dw:dw + W],
                        start=False, stop=(dw == 2),
                    )
                nc.vector.tensor_copy(stg[:, l0:l0 + 4, :], ps[:, :, :])
            nc.sync.dma_start(o_d[img, :, h0:h0 + S, :], stg[:, :, :])

        tasks = [(s, img) for s in range(NS) for img in range(B_LOC)]
        cur = load_task(tasks[0])
        for i, t in enumerate(tasks):
            nxt = load_task(tasks[i + 1]) if i + 1 < len(tasks) else None
            compute(t[0], t[1], cur)
            cur = nxt

    nc.compile()
    return nc


def kernel(x: np.ndarray, weight: np.ndarray) -> np.ndarray:
    import ml_dtypes
    from concourse.bass_utils import run_bass_kernel_spmd

    if "nc" not in _cache:
        _cache["nc"] = _build_nc()
    nc = _cache["nc"]

    x = np.ascontiguousarray(np.asarray(x)).astype(ml_dtypes.bfloat16)
    w = np.ascontiguousarray(np.asarray(weight)).astype(ml_dtypes.bfloat16)

    in_maps = [
        {"x": x[i * B_LOC:(i + 1) * B_LOC], "weight": w} for i in range(N_CORES)
    ]
    res = run_bass_kernel_spmd(nc, in_maps, list(range(N_CORES)))
    return np.concatenate(
        [np.asarray(res.results[i]["out"], dtype=np.float32)
         for i in range(N_CORES)],
        axis=0,
    )


# revision 3
# speedup vs baseline: 1.0169x; 1.0169x over previous
"""Trainium2 Bass kernel for nn_CNN2DImplemented_51994874085714.

conv2d: x (16, 64, 112, 112) f32 * weight (64, 3, 3, 128) -> (16, 128, 112, 112),
3x3, pad=1, stride=1 (weight layout (C_in, kh, kw, C_out), no bias).

Sharding: data-parallel over batch - 2 images per NeuronCore on 8 cores,
weight replicated; each core computes its shard independently (no
collectives) and the host concatenates the per-core outputs.

Design (settled by interleaved same-process A/B tests on HW):
- bf16 inputs (host-converted): same 1 col/cycle PE rate as fp32r, half the
  input DMA traffic. Output stays f32 (f32->bf16 DVE drains are slow on HW;
  bf16 out bought nothing once drains moved engines). rel err ~2.5e-3.
- 6 matmuls per 4-row output block (3 row-pair taps + 3 half-empty dh=2
  taps), N=448, one PSUM bank each, 8 banks rotating. 5-matmul column-pair
  schemes lose on HW: the shifted-copy/extra-DMA traffic costs more than the
  saved matmul.
- PSUM drains on the Act engine (frees DVE), staging in f32, one packed
  store per 28-row strip.
- First input DMA issues before the weight load (one fat wall DMA + DVE
  peel instead of 9 small SP DMAs); input DMAs run 2 tasks ahead; the last
  strip's store is split so only a 4-row tail remains at the end.
"""

from contextlib import ExitStack

import numpy as np

N_CORES = 8
B, C, H, W, O = 16, 64, 112, 112, 128
B_LOC = B // N_CORES
S = 28  # output rows per strip

_cache = {}


def _build_nc():
    import concourse.mybir as mybir
    import concourse.tile as tile
    from concourse import bacc

    BF16 = mybir.dt.bfloat16
    F32 = mybir.dt.float32
    COPY = mybir.ActivationFunctionType.Copy

    NS = H // S
    R = S + 3
    Wp = W + 2

    nc = bacc.Bacc("TRN2", target_bir_lowering=False, debug=False,
                   num_devices=N_CORES)
    x_d = nc.declare_dram_parameter("x", [B_LOC, C, H, W], BF16, isOutput=False)
    w_d = nc.declare_dram_parameter("weight", [C, 9, O], BF16, isOutput=False)
    o_d = nc.declare_dram_parameter("out", [B_LOC, O, H, W], F32, isOutput=True)

    with tile.TileContext(nc) as tc, ExitStack() as ctx:
        wpool = ctx.enter_context(tc.tile_pool(name="weights", bufs=1))
        xpool = ctx.enter_context(tc.tile_pool(name="xstrips", bufs=4))
        spool = ctx.enter_context(tc.tile_pool(name="staging", bufs=3))
        ppool = ctx.enter_context(tc.tile_pool(name="psum", bufs=8, space="PSUM"))

        # wall[c, t, o] = W[c, dh, dw, o] with t = 3*dh + dw
        wall = wpool.tile([64, 9, O], BF16, tag="wall")
        # wpair[dw] = [W(0,dw); W(1,dw)], wsing[dw] = [W(2,dw); 0]
        wpair = [
            wpool.tile([128, O], BF16, tag=f"wpair{dw}", name=f"wpair{dw}")
            for dw in range(3)
        ]
        wsing = [
            wpool.tile([128, O], BF16, tag=f"wsing{dw}", name=f"wsing{dw}")
            for dw in range(3)
        ]

        def load_weights():
            nc.sync.dma_start(wall[:, :, :], w_d[:, :, :])
            for dw in range(3):
                nc.vector.tensor_copy(wpair[dw][0:64, :], wall[:, dw, :])
                nc.vector.tensor_copy(wpair[dw][64:128, :], wall[:, 3 + dw, :])
                nc.vector.tensor_copy(wsing[dw][0:64, :], wall[:, 6 + dw, :])
                nc.vector.memset(wsing[dw][64:128, :], 0.0)

        def dma_in(t):
            # xb p0:64 row r = x_pad row h0+r (pad cols 0/113 zeroed)
            s, img = t
            h0 = s * S
            xb = xpool.tile([128, R, Wp], BF16, tag="xs")
            nc.vector.memset(xb[0:64, :, 0], 0.0)
            nc.vector.memset(xb[0:64, :, Wp - 1], 0.0)
            r_lo = max(0, 1 - h0)
            r_hi = min(S + 2, H - h0)
            if r_lo > 0:
                nc.vector.memset(xb[0:64, 0:r_lo, :], 0.0)
            if r_hi < S + 2:
                nc.vector.memset(xb[0:64, r_hi + 1:S + 3, :], 0.0)
            nc.sync.dma_start(
                xb[0:64, r_lo:r_hi + 1, 1:W + 1],
                x_d[img, :, h0 + r_lo - 1:h0 + r_hi, :],
            )
            return xb

        def make_frames(xb):
            # dh=1 frame: p64:128 = p0:64 shifted down one row
            nc.gpsimd.tensor_copy(xb[64:128, 0:S + 2, :], xb[0:64, 1:S + 3, :])
            return xb

        def compute(t, xb, last=False):
            s, img = t
            h0 = s * S
            stg = spool.tile([O, S, W], F32, tag="stg")
            for j in range(S // 4):
                l0 = 4 * j
                ps = ppool.tile([O, 4, W], F32, tag="ps")
                for dw in range(3):
                    nc.tensor.matmul(
                        ps[:, :, :], wpair[dw][:, :],
                        xb[:, l0:l0 + 4, dw:dw + W],
                        start=(dw == 0), stop=False,
                    )
                for dw in range(3):
                    nc.tensor.matmul(
                        ps[:, :, :], wsing[dw][:, :],
                        xb[:, l0 + 2:l0 + 6, dw:dw + W],
                        start=False, stop=(dw == 2),
                    )
                if last and j == S // 4 - 1:
                    # store the bulk early so only a 4-row tail remains
                    nc.sync.dma_start(o_d[img, :, h0:h0 + S - 4, :],
                                      stg[:, 0:S - 4, :])
                nc.scalar.activation(stg[:, l0:l0 + 4, :], ps[:, :, :], COPY)
            if last:
                nc.sync.dma_start(o_d[img, :, h0 + S - 4:h0 + S, :],
                                  stg[:, S - 4:S, :])
            else:
                nc.sync.dma_start(o_d[img, :, h0:h0 + S, :], stg[:, :, :])

        tasks = [(s, img) for s in range(NS) for img in range(B_LOC)]
        NT = len(tasks)

        xins = {0: dma_in(tasks[0])}
        load_weights()
        xins[1] = dma_in(tasks[1])
        frames = make_frames(xins.pop(0))
        for i, t in enumerate(tasks):
            if i + 2 < NT:
                xins[i + 2] = dma_in(tasks[i + 2])
            nxt = make_frames(xins.pop(i + 1)) if i + 1 < NT else None
            compute(t, frames, last=(i + 1 == NT))
            frames = nxt

    nc.compile()
    return nc


def kernel(x: np.ndarray, weight: np.ndarray) -> np.ndarray:
    import ml_dtypes
    from concourse.bass_utils import run_bass_kernel_spmd

    if "nc" not in _cache:
        _cache["nc"] = _build_nc()
    nc = _cache["nc"]

    x = np.ascontiguousarray(np.asarray(x)).astype(ml_dtypes.bfloat16)
    w = np.ascontiguousarray(np.asarray(weight)).reshape(C, 9, O).astype(
        ml_dtypes.bfloat16)

    in_maps = [
        {"x": x[i * B_LOC:(i + 1) * B_LOC], "weight": w} for i in range(N_CORES)
    ]
    res = run_bass_kernel_spmd(nc, in_maps, list(range(N_CORES)))
    return np.concatenate(
        [np.asarray(res.results[i]["out"], dtype=np.float32)
         for i in range(N_CORES)],
        axis=0,
    )
